# revision 1
# baseline (speedup 1.0000x reference)
"""Trainium2 Bass kernel for nn_EntityResolution (segment_reduce).

Strategy (8 cores, single launch, one AllReduce):
  - The 307MB embedding table is row-sharded: core k holds rows
    [k*12500, (k+1)*12500) of emb_weight, pre-transposed on host to
    wt = W.T shard [768, 12500].
  - Phase 1: V_k[t, c] = sum_e wt[e, t] * spansT[e, c] for all 128
    columns c = b*16 + s  (fp32r matmul, PE-transposed, written to DRAM
    as [12500, 128] rows).
  - Phase 2: every element (any batch) whose triplet id falls in shard k
    is processed on core k. Host assigns each element a slot
    (p = b*16 + m%16, i) and ships: gidx (int16 local row ids, wrapped
    for dma_gather) and satt (att value one-hot over j2 = m//16).
    dma_gather pulls 512B V rows; an identity-mask multiply+reduce
    extracts the diagonal V[lid, p]; a second multiply+reduce against
    satt yields partial sum1 [128 (b,s), 32 (j2)].
  - Phase 3: 16KB AllReduce combines the 8 partial sums.
  - Phase 4: softmax over s' (= m//32, a strided free-axis view),
    span-score multiply (hosted-mask matmul), own-batch extraction
    (hosted one-hot matmul), 512-softmax, duplicate-entity resolution
    (is_equal compare matrix), and the 1M-entity softmax emitted as a
    constant fill plus 512 scattered values.
"""
import os
import sys
sys.path.insert(0, '/opt/trn_rl_repo')

import numpy as np

import concourse.bass as bass
import concourse.bacc as bacc
import concourse.mybir as mybir
import concourse.tile as tile
from concourse import library_config
from concourse.masks import make_identity
from concourse.bass_utils import run_bass_kernel_spmd

# problem shapes (hardcoded; kernel.py must be self-contained)
B, S, C, PB, E = 8, 16, 32, 64, 768
M = S * C                # 512 bags per batch
L = M * PB               # 32768 triplet ids per batch
T = 100000               # triplet vocab
NE = 1000000             # entities
N_CORES = 8
TS = T // N_CORES        # 12500 shard rows
HALF = 6500              # v table split (multiple of 500)
NCH = 16                 # gather chunks
OUT_W = 7824             # out [128, 7824] -> flat 1001472 >= NE+1
FILL_W = OUT_W // 4

AX = mybir.AxisListType
OP = mybir.AluOpType
ACT = mybir.ActivationFunctionType
dt = mybir.dt

_cache = {}


def _build(nslot):
    phase = float(os.environ.get("K_PHASE", "9"))
    chi = nslot // NCH
    nidx = chi * 128                 # idxs per dma_gather chunk
    gw = nidx // 16                  # wrapped idx width per chunk
    nc = bacc.Bacc("TRN2", target_bir_lowering=False, debug=False,
                   num_devices=N_CORES)

    wt = nc.dram_tensor("wt", [E, TS], dt.float32, kind="ExternalInput")
    spansT = nc.dram_tensor("spansT", [E, 128], dt.float32, kind="ExternalInput")
    spans_all = nc.dram_tensor("spans_all", [128, E], dt.float32, kind="ExternalInput")
    spanw = nc.dram_tensor("spanw", [128, E], dt.float32, kind="ExternalInput")
    spanb = nc.dram_tensor("spanb", [128, 1], dt.float32, kind="ExternalInput")
    gidx = nc.dram_tensor("gidx", [128, NCH * gw], dt.int16, kind="ExternalInput")
    satt = nc.dram_tensor("satt", [128, NCH * 32 * chi], dt.float32,
                          kind="ExternalInput")
    hostb = nc.dram_tensor("hostb", [128, 128], dt.float32, kind="ExternalInput")
    hostm = nc.dram_tensor("hostm", [128, 32], dt.float32, kind="ExternalInput")
    hostown = nc.dram_tensor("hostown", [128, 16], dt.float32, kind="ExternalInput")
    qidp_i = nc.dram_tensor("qidp_i", [128, 4], dt.int32, kind="ExternalInput")
    qidp_f = nc.dram_tensor("qidp_f", [128, 4], dt.float32, kind="ExternalInput")
    qidf_free = nc.dram_tensor("qidf_free", [128, 512], dt.float32,
                               kind="ExternalInput")
    out = nc.dram_tensor("out", [128, OUT_W], dt.float32, kind="ExternalOutput")

    rg = [list(range(N_CORES))]

    with tile.TileContext(nc) as tc:
        with (
            tc.tile_pool(name="wtp", bufs=18) as wtp,
            tc.tile_pool(name="vtp", bufs=4) as vtp,
            tc.tile_pool(name="gp", bufs=2) as gp,
            tc.tile_pool(name="t2p", bufs=1) as t2p,
            tc.tile_pool(name="spp", bufs=2) as spp,
            tc.tile_pool(name="sb", bufs=1) as sb,
            tc.tile_pool(name="sm", bufs=1) as sm,
            tc.tile_pool(name="vps", bufs=4, space="PSUM") as vps,
            tc.tile_pool(name="tps", bufs=2, space="PSUM") as tps,
            tc.tile_pool(name="mps", bufs=1, space="PSUM") as mps,
            tc.tile_pool(name="dram", bufs=1, space="DRAM") as dram,
        ):
            nc.gpsimd.load_library(library_config.mlp)
            ident = sb.tile([128, 128], dt.float32)
            make_identity(nc, ident[:])

            # resident small inputs
            spansT_sb = sb.tile([128, 6, 128], dt.float32)
            for e in range(6):
                nc.sync.dma_start(spansT_sb[:, e, :], spansT[e * 128:(e + 1) * 128, :])
            gidx_sb = sb.tile([128, NCH * gw], dt.int16)
            nc.sync.dma_start(gidx_sb[:], gidx[:])

            # ---------- phase 1: V = W @ spans, PE-transposed to [t, c] ----
            # split into two tables so half-A gathers overlap half-B matmuls
            v_a = dram.tile([HALF, 128], dt.float32)
            v_b = dram.tile([TS - HALF, 128], dt.float32)
            # macro-tiles of 1000 t (8 matmul tiles of 125); last macro 500
            macs = [1000] * 12 + [500]
            moff = 0
            for mac in macs:
                wts = []
                for e in range(6):
                    w_t = wtp.tile([128, 1000], dt.float32, tag="wt")
                    nc.sync.dma_start(
                        w_t[:, :mac],
                        wt[e * 128:(e + 1) * 128, moff:moff + mac])
                    wts.append(w_t)
                for ti in range(mac // 125):
                    v_ps = vps.tile([128, 128], dt.float32)
                    for e in range(6):
                        nc.tensor.matmul(
                            v_ps[:125, :],
                            wts[e][:, ti * 125:(ti + 1) * 125],
                            spansT_sb[:, e, :],
                            start=(e == 0), stop=(e == 5))
                    vt = vtp.tile([128, 128], dt.float32, tag="vt")
                    nc.vector.tensor_copy(vt[:125, :], v_ps[:125, :])
                    row = moff + ti * 125
                    vdst = v_a if row < HALF else v_b
                    ro = row if row < HALF else row - HALF
                    nc.scalar.dma_start(vdst[ro: ro + 125, :], vt[:125, :])
                moff += mac

            if phase == 1:
                nc.sync.dma_start(
                    out[:].rearrange("p f -> (p f)")[:128 * 6000, None],
                    v_a[:6000, :].rearrange("a b -> (a b)")[:, None])

            if phase >= 2:
                # ---------- phase 2: gather + diag select + segment matrix -----
                psum1 = sb.tile([128, 32], dt.float32)
                for c in range(NCH):
                    g_t = gp.tile([128, chi, 128], dt.float32, tag="G")
                    nc.gpsimd.dma_gather(
                        out_ap=g_t[:], in_ap=(v_a if c < NCH // 2 else v_b)[:],
                        idxs_ap=gidx_sb[:, c * gw:(c + 1) * gw],
                        num_idxs=nidx, num_idxs_reg=nidx, elem_size=128,
                        single_packet=False)
                    nc.vector.tensor_tensor(
                        out=g_t[:], in0=g_t[:],
                        in1=ident[:, None, :].to_broadcast([128, chi, 128]),
                        op=OP.mult)
                    v1 = spp.tile([128, chi], dt.float32, tag="V1")
                    nc.vector.tensor_reduce(out=v1[:, :, None], in_=g_t[:],
                                            axis=AX.X, op=OP.add)
                    sa = gp.tile([128, 32, chi], dt.float32, tag="SA")
                    nc.scalar.dma_start(
                        sa[:],
                        satt[:, c * 32 * chi:(c + 1) * 32 * chi].rearrange(
                            "p (a b) -> p a b", a=32))
                    t2 = t2p.tile([128, 32, chi], dt.float32, tag="T2")
                    nc.vector.tensor_tensor(
                        out=t2[:], in0=sa[:],
                        in1=v1[:, None, :].to_broadcast([128, 32, chi]),
                        op=OP.mult)
                    psc = spp.tile([128, 32], dt.float32, tag="PSC")
                    nc.vector.tensor_reduce(out=psc[:, :, None], in_=t2[:],
                                            axis=AX.X, op=OP.add)
                    if c == 0:
                        nc.vector.tensor_copy(psum1[:], psc[:])
                    else:
                        nc.vector.tensor_add(psum1[:], psum1[:], psc[:])

                if phase == 2:
                    nc.sync.dma_start(out[:, 0:32], psum1[:])

            # ---------- phase 3: AllReduce [128, 32] -----------------------
            if phase >= 3:
                ar_in = dram.tile([128, 32], dt.float32)
                ar_out = dram.tile([128, 32], dt.float32)
                nc.gpsimd.dma_start(ar_in[:], psum1[:])
                nc.gpsimd.collective_compute(
                    "AllReduce", OP.add, replica_groups=rg,
                    ins=[ar_in.opt()], outs=[ar_out.opt()])
                sum1 = sm.tile([128, 32], dt.float32)
                nc.gpsimd.dma_start(sum1[:], ar_out[:])

                if phase == 3:
                    nc.sync.dma_start(out[:, 0:32], sum1[:])

            # ---------- phase 4: softmaxes ---------------------------------
            if phase >= 3.4:
                # span scores: ssc[p] = span_embs[p] . span_W + b
                spal = sm.tile([128, E], dt.float32)
                spwl = sm.tile([128, E], dt.float32)
                spbl = sm.tile([128, 1], dt.float32)
                nc.sync.dma_start(spal[:], spans_all[:])
                nc.sync.dma_start(spwl[:], spanw[:])
                nc.sync.dma_start(spbl[:], spanb[:])
                tmp768 = sm.tile([128, E], dt.float32)
                nc.vector.tensor_tensor(out=tmp768[:], in0=spal[:], in1=spwl[:],
                                        op=OP.mult)
                ssc = sm.tile([128, 1], dt.float32)
                nc.vector.tensor_reduce(out=ssc[:], in_=tmp768[:], axis=AX.X,
                                        op=OP.add)
                nc.vector.tensor_add(ssc[:], ssc[:], spbl[:])

                # softmax over s' = j2//2 (strided view [128, 2, 16])
                def v216(ap):
                    return ap.rearrange("p (two s2) -> p two s2", two=2)
                mx = sm.tile([128, 2], dt.float32)
                nc.vector.tensor_reduce(out=mx[:, :, None], in_=v216(sum1[:]),
                                        axis=AX.X, op=OP.max)
                e1 = sm.tile([128, 32], dt.float32)
                nc.vector.tensor_tensor(
                    out=v216(e1[:]), in0=v216(sum1[:]),
                    in1=mx[:, :, None].to_broadcast([128, 2, 16]), op=OP.subtract)
                nc.scalar.activation(e1[:], e1[:], ACT.Exp)
                smsum = sm.tile([128, 2], dt.float32)
                nc.vector.tensor_reduce(out=smsum[:, :, None], in_=v216(e1[:]),
                                        axis=AX.X, op=OP.add)
                rsm = sm.tile([128, 2], dt.float32)
                nc.vector.reciprocal(rsm[:], smsum[:])
                nc.vector.tensor_tensor(
                    out=v216(e1[:]), in0=v216(e1[:]),
                    in1=rsm[:, :, None].to_broadcast([128, 2, 16]), op=OP.mult)

                # SSB[p, j2] = span_score(b(p), j2//2) via hosted-mask matmul
                hb = sm.tile([128, 128], dt.float32)
                hm = sm.tile([128, 32], dt.float32)
                ho = sm.tile([128, 16], dt.float32)
                nc.sync.dma_start(hb[:], hostb[:])
                nc.sync.dma_start(hm[:], hostm[:])
                nc.sync.dma_start(ho[:], hostown[:])
                rhsb = sm.tile([128, 32], dt.float32)
                nc.vector.tensor_tensor(out=rhsb[:], in0=hm[:],
                                        in1=ssc[:].to_broadcast([128, 32]),
                                        op=OP.mult)
                ssb_ps = mps.tile([128, 32], dt.float32, tag="mm")
                nc.tensor.matmul(ssb_ps[:], hb[:], rhsb[:], start=True, stop=True)
                mult2 = sm.tile([128, 32], dt.float32)
                nc.vector.tensor_tensor(out=mult2[:], in0=e1[:], in1=ssb_ps[:],
                                        op=OP.mult)

                # own-batch extraction -> [16, 32] -> [1, 512]
                own_ps = mps.tile([16, 32], dt.float32, tag="mm")
                nc.tensor.matmul(own_ps[:], ho[:], mult2[:], start=True, stop=True)
                own = sm.tile([16, 32], dt.float32)
                nc.vector.tensor_copy(own[:], own_ps[:])
                cn = sm.tile([1, 512], dt.float32)
                nc.sync.dma_start(cn[:].rearrange("p (a bb) -> p a bb", a=16), own[:])

                # softmax over 512
                mxn = sm.tile([1, 1], dt.float32)
                nc.vector.tensor_reduce(out=mxn[:], in_=cn[:], axis=AX.X,
                                        op=OP.max, negate=True)
                e5 = sm.tile([1, 512], dt.float32)
                nc.scalar.activation(e5[:], cn[:], ACT.Exp, bias=mxn[:], scale=1.0)
                s5 = sm.tile([1, 1], dt.float32)
                nc.vector.tensor_reduce(out=s5[:], in_=e5[:], axis=AX.X, op=OP.add)
                r5 = sm.tile([1, 1], dt.float32)
                nc.vector.reciprocal(r5[:], s5[:])
                cand = sm.tile([1, 512], dt.float32)
                nc.vector.tensor_tensor(out=cand[:], in0=e5[:],
                                        in1=r5[:].to_broadcast([1, 512]), op=OP.mult)
                if phase == 3.5:
                    nc.sync.dma_start(out[0:1, 0:512], cand[:])


                if phase >= 3.6:
                    # ---------- phase 5: duplicate resolution + output -------------
                    ones128 = sm.tile([1, 128], dt.float32)
                    nc.vector.memset(ones128[:], 1.0)
                    cb_ps = mps.tile([128, 512], dt.float32, tag="mm")
                    nc.tensor.matmul(cb_ps[:], ones128[:], cand[:], start=True, stop=True)
                    candB = sm.tile([128, 512], dt.float32)
                    nc.vector.tensor_copy(candB[:], cb_ps[:])

                    qfp = sm.tile([128, 4], dt.float32)
                    qff = sm.tile([128, 512], dt.float32)
                    qip = sm.tile([128, 4], dt.int32)
                    nc.sync.dma_start(qfp[:], qidp_f[:])
                    nc.sync.dma_start(qff[:], qidf_free[:])
                    nc.sync.dma_start(qip[:], qidp_i[:])

                    eq = sm.tile([128, 4, 512], dt.float32)
                    nc.vector.tensor_tensor(
                        out=eq[:], in0=qfp[:, :, None].to_broadcast([128, 4, 512]),
                        in1=qff[:, None, :].to_broadcast([128, 4, 512]), op=OP.is_equal)
                    count = sm.tile([128, 4], dt.float32)
                    nc.vector.tensor_reduce(out=count[:, :, None], in_=eq[:],
                                            axis=AX.X, op=OP.add)
                    nc.vector.tensor_tensor(
                        out=eq[:], in0=eq[:],
                        in1=candB[:, None, :].to_broadcast([128, 4, 512]), op=OP.mult)
                    dup = sm.tile([128, 4], dt.float32)
                    nc.vector.tensor_reduce(out=dup[:, :, None], in_=eq[:],
                                            axis=AX.X, op=OP.add)

                    mask = sm.tile([128, 4], dt.float32)
                    nc.vector.tensor_scalar(out=mask[:], in0=qfp[:],
                                            scalar1=float(NE), scalar2=None,
                                            op0=OP.is_lt)
                    rc = sm.tile([128, 4], dt.float32)
                    nc.vector.reciprocal(rc[:], count[:])
                    mrc = sm.tile([128, 4], dt.float32)
                    nc.vector.tensor_tensor(out=mrc[:], in0=mask[:], in1=rc[:], op=OP.mult)
                    md = sm.tile([128, 4], dt.float32)
                    nc.vector.tensor_tensor(out=md[:], in0=dup[:], in1=mask[:], op=OP.mult)
                    vmp = sm.tile([128, 1], dt.float32)
                    nc.vector.tensor_reduce(out=vmp[:], in_=md[:], axis=AX.X, op=OP.max)
                    nep = sm.tile([128, 1], dt.float32)
                    nc.vector.tensor_reduce(out=nep[:], in_=mrc[:], axis=AX.X, op=OP.add)

                    # cross-partition reductions via PE transpose
                    tv_ps = tps.tile([128, 128], dt.float32, tag="tp")
                    nc.tensor.transpose(tv_ps[:1, :], vmp[:], ident[:])
                    tv = sm.tile([1, 128], dt.float32)
                    nc.vector.tensor_copy(tv[:], tv_ps[:1, :])
                    vmn = sm.tile([1, 1], dt.float32)
                    nc.vector.tensor_reduce(out=vmn[:], in_=tv[:], axis=AX.X,
                                            op=OP.max, negate=True)   # -vmax
                    tn_ps = tps.tile([128, 128], dt.float32, tag="tp")
                    nc.tensor.transpose(tn_ps[:1, :], nep[:], ident[:])
                    tn = sm.tile([1, 128], dt.float32)
                    nc.vector.tensor_copy(tn[:], tn_ps[:1, :])
                    neff = sm.tile([1, 1], dt.float32)
                    nc.vector.tensor_reduce(out=neff[:], in_=tn[:], axis=AX.X, op=OP.add)

                    vmn_ps = mps.tile([128, 1], dt.float32, tag="mm")
                    nc.tensor.matmul(vmn_ps[:], ones128[:], vmn[:], start=True, stop=True)
                    vmnB = sm.tile([128, 1], dt.float32)
                    nc.vector.tensor_copy(vmnB[:], vmn_ps[:])
                    exd = sm.tile([128, 4], dt.float32)
                    nc.scalar.activation(exd[:], dup[:], ACT.Exp, bias=vmnB[:], scale=1.0)

                    sede = sm.tile([128, 4], dt.float32)
                    nc.vector.tensor_tensor(out=sede[:], in0=mrc[:], in1=exd[:], op=OP.mult)
                    sedp = sm.tile([128, 1], dt.float32)
                    nc.vector.tensor_reduce(out=sedp[:], in_=sede[:], axis=AX.X, op=OP.add)
                    ts_ps = tps.tile([128, 128], dt.float32, tag="tp")
                    nc.tensor.transpose(ts_ps[:1, :], sedp[:], ident[:])
                    tsed = sm.tile([1, 128], dt.float32)
                    nc.vector.tensor_copy(tsed[:], ts_ps[:1, :])
                    sed0 = sm.tile([1, 1], dt.float32)
                    nc.vector.tensor_reduce(out=sed0[:], in_=tsed[:], axis=AX.X, op=OP.add)

                    e_nm = sm.tile([1, 1], dt.float32)
                    nc.scalar.activation(e_nm[:], vmn[:], ACT.Exp)     # exp(-vmax)
                    t1 = sm.tile([1, 1], dt.float32)
                    nc.vector.tensor_scalar(out=t1[:], in0=neff[:], scalar1=-1.0,
                                            scalar2=float(NE), op0=OP.mult, op1=OP.add)
                    d1 = sm.tile([1, 1], dt.float32)
                    nc.vector.tensor_tensor(out=d1[:], in0=t1[:], in1=e_nm[:], op=OP.mult)
                    denom = sm.tile([1, 1], dt.float32)
                    nc.vector.tensor_add(denom[:], d1[:], sed0[:])
                    rden = sm.tile([1, 1], dt.float32)
                    nc.vector.reciprocal(rden[:], denom[:])
                    base = sm.tile([1, 1], dt.float32)
                    nc.vector.tensor_tensor(out=base[:], in0=e_nm[:], in1=rden[:],
                                            op=OP.mult)

                    br2 = sm.tile([1, 2], dt.float32)
                    nc.vector.tensor_copy(br2[:, 0:1], rden[:])
                    nc.vector.tensor_copy(br2[:, 1:2], base[:])
                    bb_ps = mps.tile([128, 2], dt.float32, tag="mm")
                    nc.tensor.matmul(bb_ps[:], ones128[:], br2[:], start=True, stop=True)
                    bb2 = sm.tile([128, 2], dt.float32)
                    nc.vector.tensor_copy(bb2[:], bb_ps[:])

                    outv = sm.tile([128, 4], dt.float32)
                    nc.vector.tensor_tensor(out=outv[:], in0=exd[:],
                                            in1=bb2[:, 0:1].to_broadcast([128, 4]),
                                            op=OP.mult)
                    if phase == 3.8:
                        nc.sync.dma_start(out[:, 0:4], outv[:])

                if phase >= 3.9:
                    fill = sm.tile([128, FILL_W], dt.float32)
                    nc.vector.tensor_copy(fill[:],
                                          bb2[:, 1:2].to_broadcast([128, FILL_W]))
                    for q in range(4):
                        nc.sync.dma_start(out[:, q * FILL_W:(q + 1) * FILL_W], fill[:])
                    tc.strict_bb_all_engine_barrier()
                    out_flat = out[:].rearrange("p f -> (p f)")[:, None]
                    for q in range(4):
                        nc.gpsimd.indirect_dma_start(
                            out=out_flat,
                            out_offset=bass.IndirectOffsetOnAxis(ap=qip[:, q:q + 1], axis=0),
                            in_=outv[:, q:q + 1],
                            in_offset=None)

    nc.compile()
    return nc


def _host_prep(span_embs, triplet_ids_tr, offsets_tr, attention_tr, qid_inds,
               emb_weight, span_W, span_b):
    span_embs = np.asarray(span_embs, dtype=np.float32)
    ids = np.asarray(triplet_ids_tr).astype(np.int64)
    offs = np.asarray(offsets_tr).astype(np.int64)
    att = np.asarray(attention_tr, dtype=np.float32)
    qid = np.asarray(qid_inds).astype(np.int64)
    emb_weight = np.asarray(emb_weight, dtype=np.float32)
    span_W = np.asarray(span_W, dtype=np.float32)
    span_b = np.asarray(span_b, dtype=np.float32)

    # bag id per element (general sorted offsets, offs[b,0] == 0)
    pos = np.arange(L)
    seg = np.empty((B, L), dtype=np.int64)
    for b in range(B):
        seg[b] = np.searchsorted(offs[b], pos, side='right') - 1

    bcol = (np.arange(B)[:, None] * 16 + (seg % 16))        # p = b*16 + m%16
    # device j2 axis: groups contiguous for the s'-softmax
    j2 = ((seg // 16) % 2) * 16 + seg // 32
    k_of = ids // TS
    lid = (ids % TS).astype(np.int64)
    halfsel = (lid >= HALF).astype(np.int64)
    lidx = lid - HALF * halfsel

    # rank within (core, half, partition) group, in stable order
    key = ((k_of * 2 + halfsel) * 128 + bcol).ravel()
    order = np.argsort(key, kind='stable')
    sk = key[order]
    starts = np.r_[0, np.flatnonzero(sk[1:] != sk[:-1]) + 1]
    group_id = np.cumsum(np.r_[0, (sk[1:] != sk[:-1]).astype(np.int64)])
    rank_sorted = np.arange(sk.size) - starts[group_id]
    rank = np.empty(sk.size, dtype=np.int64)
    rank[order] = rank_sorted

    max_rank = int(rank.max())
    nhalf = max(192, ((max_rank + 1 + 7) // 8) * 8)   # per-half slots
    nslot = 2 * nhalf
    chi = nslot // NCH
    gw = chi * 128 // 16

    kf = k_of.ravel()
    pf = bcol.ravel()
    j2f = j2.ravel()
    lf = lidx.ravel()
    af = att.ravel().astype(np.float32)
    cf = halfsel.ravel() * (NCH // 2) + rank // chi   # chunk
    ilocf = rank % chi

    # spansT [768, 128] (col = b*16+s) and spans_all [128, 768]
    spans_all = np.ascontiguousarray(span_embs.reshape(128, E))
    spansT = np.ascontiguousarray(spans_all.T)
    WT = np.ascontiguousarray(emb_weight.T)          # [768, 100000]
    spanw = np.tile(span_W[:, 0][None, :], (128, 1)).astype(np.float32)
    spanb_r = np.full((128, 1), float(span_b[0]), dtype=np.float32)

    r = np.arange(128)
    hostb = (r[:, None] // 16 == r[None, :] // 16).astype(np.float32)
    hostm = (r[:, None] % 16 == np.arange(32)[None, :] % 16).astype(np.float32)

    x = np.arange(512)
    j2d = x % 32
    mx_map = x // 32 + 16 * (2 * (j2d % 16) + j2d // 16)   # position x -> bag m

    in_maps = []
    for k in range(N_CORES):
        sel = kf == k
        p_k, j2_k = pf[sel], j2f[sel]
        l_k, a_k = lf[sel], af[sel]
        c_k, il_k = cf[sel], ilocf[sel]

        gidx_flat = np.zeros((NCH, chi * 128), dtype=np.int16)
        gidx_flat[c_k, il_k * 128 + p_k] = l_k.astype(np.int16)
        gidx = np.zeros((128, NCH * gw), dtype=np.int16)
        for c in range(NCH):
            wrapped = gidx_flat[c].reshape(gw, 16).T       # [16, gw]
            gidx[:, c * gw:(c + 1) * gw] = np.tile(wrapped, (8, 1))

        satt = np.zeros((128, NCH, 32, chi), dtype=np.float32)
        satt[p_k, c_k, j2_k, il_k] = a_k

        own = k
        hostown = np.zeros((128, 16), dtype=np.float32)
        hostown[own * 16 + np.arange(16), np.arange(16)] = 1.0

        qx = qid[own][mx_map]
        in_maps.append(dict(
            wt=np.ascontiguousarray(WT[:, k * TS:(k + 1) * TS]),
            spansT=spansT, spans_all=spans_all, spanw=spanw, spanb=spanb_r,
            gidx=gidx, satt=np.ascontiguousarray(satt.reshape(128, -1)),
            hostb=hostb, hostm=hostm, hostown=hostown,
            qidp_i=qx.reshape(128, 4).astype(np.int32),
            qidp_f=qx.reshape(128, 4).astype(np.float32),
            qidf_free=np.tile(qx[None, :], (128, 1)).astype(np.float32),
        ))
    return in_maps, nslot


def kernel_run(inputs, trace=False):
    in_maps, nslot = _host_prep(**inputs)
    if nslot not in _cache:
        _cache[nslot] = _build(nslot)
    nc = _cache[nslot]
    res = run_bass_kernel_spmd(nc, in_maps, core_ids=list(range(N_CORES)),
                               trace=trace)
    out = np.stack([r["out"].reshape(-1)[:NE] for r in res.results])
    return out[:, :, None].astype(np.float32), res


def kernel(**inputs):
    out, _ = kernel_run(inputs)
    return out



# revision 2
# speedup vs baseline: 1.1087x; 1.1087x over previous
"""Trainium2 Bass kernel for nn_EntityResolution (segment_reduce).

Strategy (8 cores, single launch, one 16KB AllReduce):
  - The triplet table is row-sharded: core k holds rows [k*12500, (k+1)*12500)
    of emb_weight, shipped as bf16 W^T tiles pre-arranged for streaming.
  - Phase 1: Vt[p, t] = sum_e spansT[e, p] * wt[e, t] on the PE
    (spansT chunks stationary, wt streamed in N=500 bf16 matmuls),
    accumulated in PSUM and copied to a resident SBUF table
    Vt [128, 12500] fp32 -- V never touches DRAM.
  - Phase 2: gpsimd ap_gather pulls Vt[p, lid] for every triplet element.
    GPSIMD core g serves partitions [16g, 16g+16) = batch g's 16 span
    columns, so one shared per-core index list (host-sorted by j2 bucket,
    padded to NJ slots per bucket) gathers all of batch g's elements.
    A hosted mask (att folded in, one-hot over the span column s=m%16)
    multiplies the gather output and a single free-axis reduce produces
    the partial sum1 [128 (b,s), 32 (j2)].
  - Phase 3: 16KB AllReduce combines the 8 partial sums.
  - Phase 4: softmax over s' (strided free-axis view), span-score multiply
    (hosted-mask matmul), own-batch extraction, 512-softmax.
  - Phase 5: duplicate-entity resolution (is_equal compare matrix) and the
    1M-entity softmax emitted as a constant fill plus 512 scattered values.
"""
import sys
sys.path.insert(0, '/opt/trn_rl_repo')

import numpy as np

import concourse.bass as bass
import concourse.bacc as bacc
import concourse.mybir as mybir
import concourse.tile as tile
from concourse import library_config
from concourse.masks import make_identity
from concourse.bass_utils import run_bass_kernel_spmd

# problem shapes (hardcoded; kernel.py must be self-contained)
B, S, C, PB, E = 8, 16, 32, 64, 768
M = S * C                # 512 bags per batch
L = M * PB               # 32768 triplet ids per batch
T = 100000               # triplet vocab
NE = 1000000             # entities
N_CORES = 8
TS = T // N_CORES        # 12500 shard rows
NTILE = 25               # phase-1 t tiles
TC = TS // NTILE         # 500 t-cols per tile
OUT_W = 7824             # out [128, 7824] -> flat 1001472 >= NE+1
FILL_W = OUT_W // 4

# aux (f32 [128, AUXW]) block offsets
OFF_SPAL = 0             # span_embs rows              (768)
OFF_SPW = 768            # span_W broadcast            (768)
OFF_SPB = 1536           # span_b                      (1)
OFF_HB = 1537            # same-batch mask             (128)
OFF_HM = 1665            # span-col one-hot            (32)
OFF_HO = 1697            # own-batch extraction        (16)
OFF_QF = 1713            # qid (f32, partition's 4)    (4)
OFF_QFF = 1717           # qid full list               (512)
OFF_MATT = 2240          # gather mask * att           (NI)

AX = mybir.AxisListType
OP = mybir.AluOpType
ACT = mybir.ActivationFunctionType
dt = mybir.dt

_cache = {}


def _build(NJ):
    NI = 32 * NJ                    # gather slots per gpsimd core
    AUXW = OFF_MATT + NI
    nc = bacc.Bacc("TRN2", target_bir_lowering=False, debug=False,
                   num_devices=N_CORES)

    wb = nc.dram_tensor("wb", [128, NTILE * 6 * TC + 768], dt.bfloat16,
                        kind="ExternalInput")
    aux = nc.dram_tensor("aux", [128, AUXW], dt.float32, kind="ExternalInput")
    gidx = nc.dram_tensor("gidx", [128, NI // 16], dt.int16,
                          kind="ExternalInput")
    qidp_i = nc.dram_tensor("qidp_i", [128, 4], dt.int32, kind="ExternalInput")
    out = nc.dram_tensor("out", [128, OUT_W], dt.float32, kind="ExternalOutput")

    rg = [list(range(N_CORES))]

    with tile.TileContext(nc) as tc:
        with (
            tc.tile_pool(name="wbp", bufs=3) as wbp,
            tc.tile_pool(name="sb", bufs=1) as sb,
            tc.tile_pool(name="sm", bufs=1) as sm,
            tc.tile_pool(name="vps", bufs=4, space="PSUM") as vps,
            tc.tile_pool(name="tps", bufs=2, space="PSUM") as tps,
            tc.tile_pool(name="mps", bufs=1, space="PSUM") as mps,
            tc.tile_pool(name="dram", bufs=1, space="DRAM") as dram,
        ):
            nc.gpsimd.load_library(library_config.ap_gather)
            ident = sb.tile([128, 128], dt.float32)
            make_identity(nc, ident[:])

            # resident small inputs
            spansT_sb = sb.tile([128, 6, 128], dt.bfloat16)
            nc.sync.dma_start(
                spansT_sb[:],
                wb[:, NTILE * 6 * TC:].rearrange("p (a b) -> p a b", a=6))
            gidx_sb = sb.tile([128, NI // 16], dt.int16)
            nc.sync.dma_start(gidx_sb[:], gidx[:])
            aux_sb = sb.tile([128, AUXW], dt.float32)
            nc.sync.dma_start(aux_sb[:], aux[:])

            # ---------- phase 1: Vt[p, t] = sum_e spansT[e, p] wt[e, t] ----
            vt = sb.tile([128, TS], dt.float32)
            for tau in range(NTILE):
                w_t = wbp.tile([128, 6 * TC], dt.bfloat16, tag="wt")
                nc.sync.dma_start(
                    w_t[:], wb[:, tau * 6 * TC:(tau + 1) * 6 * TC])
                ps = vps.tile([128, TC], dt.float32)
                for e in range(6):
                    nc.tensor.matmul(
                        ps[:], spansT_sb[:, e, :],
                        w_t[:, e * TC:(e + 1) * TC],
                        start=(e == 0), stop=(e == 5))
                nc.vector.tensor_copy(vt[:, tau * TC:(tau + 1) * TC], ps[:])

            # ---------- phase 2: gather + mask + bucket reduce -------------
            g = sb.tile([128, NI], dt.float32)
            nc.gpsimd.ap_gather(
                out_ap=g[:], in_ap=vt[:], idxs_ap=gidx_sb[:],
                channels=128, num_elems=TS, d=1, num_idxs=NI)
            nc.vector.tensor_tensor(
                out=g[:], in0=g[:], in1=aux_sb[:, OFF_MATT:OFF_MATT + NI],
                op=OP.mult)
            psum1 = sb.tile([128, 32], dt.float32)
            nc.vector.tensor_reduce(
                out=psum1[:, :, None],
                in_=g[:].rearrange("p (a b) -> p a b", a=32),
                axis=AX.X, op=OP.add)

            # ---------- phase 3: AllReduce [128, 32] -----------------------
            ar_in = dram.tile([128, 32], dt.float32)
            ar_out = dram.tile([128, 32], dt.float32)
            nc.gpsimd.dma_start(ar_in[:], psum1[:])
            nc.gpsimd.collective_compute(
                "AllReduce", OP.add, replica_groups=rg,
                ins=[ar_in.opt()], outs=[ar_out.opt()])
            sum1 = sm.tile([128, 32], dt.float32)
            nc.gpsimd.dma_start(sum1[:], ar_out[:])

            # ---------- phase 4: softmaxes ---------------------------------
            # span scores: ssc[p] = span_embs[p] . span_W + b
            tmp768 = sm.tile([128, E], dt.float32)
            nc.vector.tensor_tensor(
                out=tmp768[:], in0=aux_sb[:, OFF_SPAL:OFF_SPAL + E],
                in1=aux_sb[:, OFF_SPW:OFF_SPW + E], op=OP.mult)
            ssc = sm.tile([128, 1], dt.float32)
            nc.vector.tensor_reduce(out=ssc[:], in_=tmp768[:], axis=AX.X,
                                    op=OP.add)
            nc.vector.tensor_add(ssc[:], ssc[:],
                                 aux_sb[:, OFF_SPB:OFF_SPB + 1])

            # softmax over s' = j2 % 16 (strided view [128, 2, 16])
            def v216(ap):
                return ap.rearrange("p (two s2) -> p two s2", two=2)
            mx = sm.tile([128, 2], dt.float32)
            nc.vector.tensor_reduce(out=mx[:, :, None], in_=v216(sum1[:]),
                                    axis=AX.X, op=OP.max)
            e1 = sm.tile([128, 32], dt.float32)
            nc.vector.tensor_tensor(
                out=v216(e1[:]), in0=v216(sum1[:]),
                in1=mx[:, :, None].to_broadcast([128, 2, 16]), op=OP.subtract)
            nc.scalar.activation(e1[:], e1[:], ACT.Exp)
            smsum = sm.tile([128, 2], dt.float32)
            nc.vector.tensor_reduce(out=smsum[:, :, None], in_=v216(e1[:]),
                                    axis=AX.X, op=OP.add)
            rsm = sm.tile([128, 2], dt.float32)
            nc.vector.reciprocal(rsm[:], smsum[:])
            nc.vector.tensor_tensor(
                out=v216(e1[:]), in0=v216(e1[:]),
                in1=rsm[:, :, None].to_broadcast([128, 2, 16]), op=OP.mult)

            # SSB[p, j2] = span_score(b(p), j2 % 16) via hosted-mask matmul
            rhsb = sm.tile([128, 32], dt.float32)
            nc.vector.tensor_tensor(out=rhsb[:],
                                    in0=aux_sb[:, OFF_HM:OFF_HM + 32],
                                    in1=ssc[:].to_broadcast([128, 32]),
                                    op=OP.mult)
            ssb_ps = mps.tile([128, 32], dt.float32, tag="mm")
            nc.tensor.matmul(ssb_ps[:], aux_sb[:, OFF_HB:OFF_HB + 128],
                             rhsb[:], start=True, stop=True)
            mult2 = sm.tile([128, 32], dt.float32)
            nc.vector.tensor_tensor(out=mult2[:], in0=e1[:], in1=ssb_ps[:],
                                    op=OP.mult)

            # own-batch extraction -> [16, 32] -> [1, 512]
            own_ps = mps.tile([16, 32], dt.float32, tag="mm")
            nc.tensor.matmul(own_ps[:], aux_sb[:, OFF_HO:OFF_HO + 16],
                             mult2[:], start=True, stop=True)
            own = sm.tile([16, 32], dt.float32)
            nc.vector.tensor_copy(own[:], own_ps[:])
            cn = sm.tile([1, 512], dt.float32)
            nc.sync.dma_start(cn[:].rearrange("p (a bb) -> p a bb", a=16),
                              own[:])

            # softmax over 512
            mxn = sm.tile([1, 1], dt.float32)
            nc.vector.tensor_reduce(out=mxn[:], in_=cn[:], axis=AX.X,
                                    op=OP.max, negate=True)
            e5 = sm.tile([1, 512], dt.float32)
            nc.scalar.activation(e5[:], cn[:], ACT.Exp, bias=mxn[:], scale=1.0)
            s5 = sm.tile([1, 1], dt.float32)
            nc.vector.tensor_reduce(out=s5[:], in_=e5[:], axis=AX.X, op=OP.add)
            r5 = sm.tile([1, 1], dt.float32)
            nc.vector.reciprocal(r5[:], s5[:])
            cand = sm.tile([1, 512], dt.float32)
            nc.vector.tensor_tensor(out=cand[:], in0=e5[:],
                                    in1=r5[:].to_broadcast([1, 512]),
                                    op=OP.mult)

            # ---------- phase 5: duplicate resolution + output -------------
            ones128 = sm.tile([1, 128], dt.float32)
            nc.vector.memset(ones128[:], 1.0)
            cb_ps = mps.tile([128, 512], dt.float32, tag="mm")
            nc.tensor.matmul(cb_ps[:], ones128[:], cand[:], start=True,
                             stop=True)
            candB = sm.tile([128, 512], dt.float32)
            nc.vector.tensor_copy(candB[:], cb_ps[:])

            qfp = aux_sb[:, OFF_QF:OFF_QF + 4]
            qff = aux_sb[:, OFF_QFF:OFF_QFF + 512]
            qip = sm.tile([128, 4], dt.int32)
            nc.sync.dma_start(qip[:], qidp_i[:])

            eq = sm.tile([128, 4, 512], dt.float32)
            nc.vector.tensor_tensor(
                out=eq[:], in0=qfp[:, :, None].to_broadcast([128, 4, 512]),
                in1=qff[:, None, :].to_broadcast([128, 4, 512]),
                op=OP.is_equal)
            count = sm.tile([128, 4], dt.float32)
            nc.vector.tensor_reduce(out=count[:, :, None], in_=eq[:],
                                    axis=AX.X, op=OP.add)
            nc.vector.tensor_tensor(
                out=eq[:], in0=eq[:],
                in1=candB[:, None, :].to_broadcast([128, 4, 512]), op=OP.mult)
            dup = sm.tile([128, 4], dt.float32)
            nc.vector.tensor_reduce(out=dup[:, :, None], in_=eq[:],
                                    axis=AX.X, op=OP.add)

            mask = sm.tile([128, 4], dt.float32)
            nc.vector.tensor_scalar(out=mask[:], in0=qfp[:],
                                    scalar1=float(NE), scalar2=None,
                                    op0=OP.is_lt)
            rc = sm.tile([128, 4], dt.float32)
            nc.vector.reciprocal(rc[:], count[:])
            mrc = sm.tile([128, 4], dt.float32)
            nc.vector.tensor_tensor(out=mrc[:], in0=mask[:], in1=rc[:],
                                    op=OP.mult)
            md = sm.tile([128, 4], dt.float32)
            nc.vector.tensor_tensor(out=md[:], in0=dup[:], in1=mask[:],
                                    op=OP.mult)
            vmp = sm.tile([128, 1], dt.float32)
            nc.vector.tensor_reduce(out=vmp[:], in_=md[:], axis=AX.X,
                                    op=OP.max)
            nep = sm.tile([128, 1], dt.float32)
            nc.vector.tensor_reduce(out=nep[:], in_=mrc[:], axis=AX.X,
                                    op=OP.add)

            # cross-partition reductions via PE transpose
            tv_ps = tps.tile([128, 128], dt.float32, tag="tp")
            nc.tensor.transpose(tv_ps[:1, :], vmp[:], ident[:])
            tv = sm.tile([1, 128], dt.float32)
            nc.vector.tensor_copy(tv[:], tv_ps[:1, :])
            vmn = sm.tile([1, 1], dt.float32)
            nc.vector.tensor_reduce(out=vmn[:], in_=tv[:], axis=AX.X,
                                    op=OP.max, negate=True)   # -vmax
            tn_ps = tps.tile([128, 128], dt.float32, tag="tp")
            nc.tensor.transpose(tn_ps[:1, :], nep[:], ident[:])
            tn = sm.tile([1, 128], dt.float32)
            nc.vector.tensor_copy(tn[:], tn_ps[:1, :])
            neff = sm.tile([1, 1], dt.float32)
            nc.vector.tensor_reduce(out=neff[:], in_=tn[:], axis=AX.X,
                                    op=OP.add)

            vmn_ps = mps.tile([128, 1], dt.float32, tag="mm")
            nc.tensor.matmul(vmn_ps[:], ones128[:], vmn[:], start=True,
                             stop=True)
            vmnB = sm.tile([128, 1], dt.float32)
            nc.vector.tensor_copy(vmnB[:], vmn_ps[:])
            exd = sm.tile([128, 4], dt.float32)
            nc.scalar.activation(exd[:], dup[:], ACT.Exp, bias=vmnB[:],
                                 scale=1.0)

            sede = sm.tile([128, 4], dt.float32)
            nc.vector.tensor_tensor(out=sede[:], in0=mrc[:], in1=exd[:],
                                    op=OP.mult)
            sedp = sm.tile([128, 1], dt.float32)
            nc.vector.tensor_reduce(out=sedp[:], in_=sede[:], axis=AX.X,
                                    op=OP.add)
            ts_ps = tps.tile([128, 128], dt.float32, tag="tp")
            nc.tensor.transpose(ts_ps[:1, :], sedp[:], ident[:])
            tsed = sm.tile([1, 128], dt.float32)
            nc.vector.tensor_copy(tsed[:], ts_ps[:1, :])
            sed0 = sm.tile([1, 1], dt.float32)
            nc.vector.tensor_reduce(out=sed0[:], in_=tsed[:], axis=AX.X,
                                    op=OP.add)

            e_nm = sm.tile([1, 1], dt.float32)
            nc.scalar.activation(e_nm[:], vmn[:], ACT.Exp)     # exp(-vmax)
            t1 = sm.tile([1, 1], dt.float32)
            nc.vector.tensor_scalar(out=t1[:], in0=neff[:], scalar1=-1.0,
                                    scalar2=float(NE), op0=OP.mult,
                                    op1=OP.add)
            d1 = sm.tile([1, 1], dt.float32)
            nc.vector.tensor_tensor(out=d1[:], in0=t1[:], in1=e_nm[:],
                                    op=OP.mult)
            denom = sm.tile([1, 1], dt.float32)
            nc.vector.tensor_add(denom[:], d1[:], sed0[:])
            rden = sm.tile([1, 1], dt.float32)
            nc.vector.reciprocal(rden[:], denom[:])
            base = sm.tile([1, 1], dt.float32)
            nc.vector.tensor_tensor(out=base[:], in0=e_nm[:], in1=rden[:],
                                    op=OP.mult)

            br2 = sm.tile([1, 2], dt.float32)
            nc.vector.tensor_copy(br2[:, 0:1], rden[:])
            nc.vector.tensor_copy(br2[:, 1:2], base[:])
            bb_ps = mps.tile([128, 2], dt.float32, tag="mm")
            nc.tensor.matmul(bb_ps[:], ones128[:], br2[:], start=True,
                             stop=True)
            bb2 = sm.tile([128, 2], dt.float32)
            nc.vector.tensor_copy(bb2[:], bb_ps[:])

            outv = sm.tile([128, 4], dt.float32)
            nc.vector.tensor_tensor(out=outv[:], in0=exd[:],
                                    in1=bb2[:, 0:1].to_broadcast([128, 4]),
                                    op=OP.mult)

            fill = sm.tile([128, FILL_W], dt.float32)
            nc.vector.tensor_copy(fill[:],
                                  bb2[:, 1:2].to_broadcast([128, FILL_W]))
            for q in range(4):
                nc.sync.dma_start(out[:, q * FILL_W:(q + 1) * FILL_W],
                                  fill[:])
            tc.strict_bb_all_engine_barrier()
            out_flat = out[:].rearrange("p f -> (p f)")[:, None]
            for q in range(4):
                nc.gpsimd.indirect_dma_start(
                    out=out_flat,
                    out_offset=bass.IndirectOffsetOnAxis(ap=qip[:, q:q + 1],
                                                         axis=0),
                    in_=outv[:, q:q + 1],
                    in_offset=None)

    nc.compile()
    return nc


def _host_prep(span_embs, triplet_ids_tr, offsets_tr, attention_tr, qid_inds,
               emb_weight, span_W, span_b):
    span_embs = np.asarray(span_embs, dtype=np.float32)
    ids = np.asarray(triplet_ids_tr).astype(np.int64)
    offs = np.asarray(offsets_tr).astype(np.int64)
    att = np.asarray(attention_tr, dtype=np.float32)
    qid = np.asarray(qid_inds).astype(np.int64)
    emb_weight = np.asarray(emb_weight, dtype=np.float32)
    span_W = np.asarray(span_W, dtype=np.float32)
    span_b = np.asarray(span_b, dtype=np.float32)
    bf16 = mybir.dt.np(mybir.dt.bfloat16)

    # bag id per element (general sorted offsets, offs[b,0] == 0)
    pos = np.arange(L)
    seg = np.empty((B, L), dtype=np.int64)
    for b in range(B):
        seg[b] = np.searchsorted(offs[b], pos, side='right') - 1

    su = seg % 16                                 # span col / channel-in-group
    j2 = ((seg // 16) % 2) * 16 + seg // 32       # bucket (contiguous softmax)
    k_of = ids // TS
    lid = (ids % TS).astype(np.int64)
    bidx = np.broadcast_to(np.arange(B)[:, None], (B, L))

    # rank within (core k, batch b, bucket j2), stable order
    key = ((k_of * B + bidx) * 32 + j2).ravel()
    order = np.argsort(key, kind='stable')
    sk = key[order]
    starts = np.r_[0, np.flatnonzero(sk[1:] != sk[:-1]) + 1]
    group_id = np.cumsum(np.r_[0, (sk[1:] != sk[:-1]).astype(np.int64)])
    rank_sorted = np.arange(sk.size) - starts[group_id]
    rank = np.empty(sk.size, dtype=np.int64)
    rank[order] = rank_sorted

    NJ = max(160, ((int(rank.max()) + 1 + 7) // 8) * 8)   # slots per bucket
    NI = 32 * NJ
    slot = (j2.ravel() * NJ + rank)

    kf = k_of.ravel()
    bf = bidx.ravel()
    gidx_all = np.zeros((N_CORES, B, NI), dtype=np.int16)
    gidx_all[kf, bf, slot] = lid.ravel().astype(np.int16)
    matt_all = np.zeros((N_CORES, B, 16, NI), dtype=np.float32)
    matt_all[kf, bf, su.ravel(), slot] = att.ravel()

    # wb: streaming W^T tiles + spansT, bf16
    WT = emb_weight.T                              # [768, 100000] f32
    spans_all = np.ascontiguousarray(span_embs.reshape(128, E))
    spansT_blk = spans_all.T.reshape(6, 128, 128).transpose(1, 0, 2) \
        .reshape(128, 768)

    spanw = np.tile(span_W[:, 0][None, :], (128, 1)).astype(np.float32)
    r = np.arange(128)
    hostb = (r[:, None] // 16 == r[None, :] // 16).astype(np.float32)
    hostm = (r[:, None] % 16 == np.arange(32)[None, :] % 16) \
        .astype(np.float32)

    x = np.arange(512)
    j2d = x % 32
    mx_map = x // 32 + 16 * (2 * (j2d % 16) + j2d // 16)   # position -> bag

    AUXW = OFF_MATT + NI
    in_maps = []
    for k in range(N_CORES):
        wbk = np.empty((128, NTILE * 6 * TC + 768), dtype=bf16)
        wtk = WT[:, k * TS:(k + 1) * TS]           # [768, 12500]
        wbk[:, :NTILE * 6 * TC] = (
            wtk.reshape(6, 128, NTILE, TC).transpose(1, 2, 0, 3)
            .reshape(128, NTILE * 6 * TC).astype(bf16))
        wbk[:, NTILE * 6 * TC:] = spansT_blk.astype(bf16)

        hostown = np.zeros((128, 16), dtype=np.float32)
        hostown[k * 16 + np.arange(16), np.arange(16)] = 1.0
        qx = qid[k][mx_map]

        auxk = np.zeros((128, AUXW), dtype=np.float32)
        auxk[:, OFF_SPAL:OFF_SPAL + E] = spans_all
        auxk[:, OFF_SPW:OFF_SPW + E] = spanw
        auxk[:, OFF_SPB] = float(span_b[0])
        auxk[:, OFF_HB:OFF_HB + 128] = hostb
        auxk[:, OFF_HM:OFF_HM + 32] = hostm
        auxk[:, OFF_HO:OFF_HO + 16] = hostown
        auxk[:, OFF_QF:OFF_QF + 4] = qx.reshape(128, 4)
        auxk[:, OFF_QFF:OFF_QFF + 512] = qx[None, :]
        auxk[:, OFF_MATT:] = matt_all[k].reshape(128, NI)

        # wrap idx j -> partition 16b + j%16, free j//16
        gk = np.zeros((128, NI // 16), dtype=np.int16)
        for b in range(B):
            gk[b * 16:(b + 1) * 16, :] = gidx_all[k, b].reshape(NI // 16, 16).T

        in_maps.append(dict(
            wb=wbk, aux=auxk, gidx=gk,
            qidp_i=qx.reshape(128, 4).astype(np.int32),
        ))
    return in_maps, NJ


def kernel_run(inputs, trace=False):
    in_maps, NJ = _host_prep(**inputs)
    if NJ not in _cache:
        _cache[NJ] = _build(NJ)
    nc = _cache[NJ]
    res = run_bass_kernel_spmd(nc, in_maps, core_ids=list(range(N_CORES)),
                               trace=trace)
    out = np.stack([r["out"].reshape(-1)[:NE] for r in res.results])
    return out[:, :, None].astype(np.float32), res


def kernel(**inputs):
    out, _ = kernel_run(inputs)
    return out


# revision 17
# speedup vs baseline: 1.1122x; 1.0032x over previous
"""Trainium2 Bass kernel for nn_EntityResolution (segment_reduce).

Strategy (8 cores, single launch, one 16KB AllReduce):
  - The triplet table is row-sharded: core k holds rows [k*12500, (k+1)*12500)
    of emb_weight, shipped as bf16 W^T tiles pre-arranged for streaming.
  - Phase 1: Vt[p, t] = sum_e spansT[e, p] * wt[e, t] on the PE
    (spansT chunks stationary, wt streamed in N=500 bf16 matmuls),
    accumulated in PSUM and copied to a resident SBUF table
    Vt [128, 12500] fp32 -- V never touches DRAM.
  - Phase 2: gpsimd ap_gather pulls Vt[p, lid] for every triplet element.
    GPSIMD core g serves partitions [16g, 16g+16) = batch g's 16 span
    columns, so one shared per-core index list (host-sorted by j2 bucket,
    padded to NJ slots per bucket) gathers all of batch g's elements.
    A hosted mask (att folded in, one-hot over the span column s=m%16)
    multiplies the gather output and a single free-axis reduce produces
    the partial sum1 [128 (b,s), 32 (j2)].
  - Phase 3: 16KB AllReduce combines the 8 partial sums.
  - Phase 4: softmax over s' (strided free-axis view), span-score multiply
    (hosted-mask matmul), own-batch extraction, 512-softmax.
  - Phase 5: duplicate-entity resolution (is_equal compare matrix) and the
    1M-entity softmax emitted as a constant fill plus 512 scattered values.
"""
import sys
sys.path.insert(0, '/opt/trn_rl_repo')

import numpy as np

import concourse.bass as bass
import concourse.bacc as bacc
import concourse.mybir as mybir
import concourse.tile as tile
from concourse import library_config
from concourse.masks import make_identity
from concourse.bass_utils import run_bass_kernel_spmd

# problem shapes (hardcoded; kernel.py must be self-contained)
B, S, C, PB, E = 8, 16, 32, 64, 768
M = S * C                # 512 bags per batch
L = M * PB               # 32768 triplet ids per batch
T = 100000               # triplet vocab
NE = 1000000             # entities
N_CORES = 8
TS = T // N_CORES        # 12500 shard rows
NTILE = 25               # phase-1 t tiles
TC = TS // NTILE         # 500 t-cols per tile
NT0 = 13                 # tiles in table half 0 (gather overlap split)
H0 = NT0 * TC            # 6500 rows in half 0
H1 = TS - H0             # 6000 rows in half 1
WSC = 32.0               # host scale on W to stay in fp8 normal range
OUT_W = 7824             # out [128, 7824] -> flat 1001472 >= NE+1
FILL_W = OUT_W // 4

# aux (f32 [128, AUXW]) block offsets
OFF_SPAL = 0             # span_embs rows              (768)
OFF_SPW = 768            # span_W broadcast            (768)
OFF_SPB = 1536           # span_b                      (1)
OFF_HB = 1537            # same-batch mask             (128)
OFF_HM = 1665            # span-col one-hot            (32)
OFF_HO = 1697            # own-batch extraction        (16)
OFF_QF = 1713            # qid (f32, partition's 4)    (4)
OFF_QFF = 1717           # qid full list               (512)
OFF_MATT = 2240          # gather mask * att / WSC     (2 * NIH)

AX = mybir.AxisListType
OP = mybir.AluOpType
ACT = mybir.ActivationFunctionType
dt = mybir.dt

_cache = {}


def _build(NJ):
    import os
    phase = float(os.environ.get("K2_PHASE", "9"))
    NIH = 32 * NJ                   # gather slots per gpsimd core per half
    AUXW = OFF_MATT + 2 * NIH
    nc = bacc.Bacc("TRN2", target_bir_lowering=False, debug=False,
                   num_devices=N_CORES)

    wb = nc.dram_tensor("wb", [128, NTILE * 6 * TC + 768], dt.float8e4,
                        kind="ExternalInput")
    aux = nc.dram_tensor("aux", [128, AUXW], dt.float32, kind="ExternalInput")
    gidx = nc.dram_tensor("gidx", [128, 2 * (NIH // 16)], dt.int16,
                          kind="ExternalInput")
    qidp_i = nc.dram_tensor("qidp_i", [128, 4], dt.int32, kind="ExternalInput")
    out = nc.dram_tensor("out", [128, OUT_W], dt.float32, kind="ExternalOutput")

    rg = [list(range(N_CORES))]

    with tile.TileContext(nc) as tc:
        with (
            tc.tile_pool(name="wbp", bufs=3) as wbp,
            tc.tile_pool(name="sb", bufs=1) as sb,
            tc.tile_pool(name="sm", bufs=1) as sm,
            tc.tile_pool(name="vps", bufs=4, space="PSUM") as vps,
            tc.tile_pool(name="tps", bufs=2, space="PSUM") as tps,
            tc.tile_pool(name="mps", bufs=1, space="PSUM") as mps,
            tc.tile_pool(name="dram", bufs=1, space="DRAM") as dram,
        ):
            nc.gpsimd.load_library(library_config.ap_gather)
            ident = sb.tile([128, 128], dt.float32)
            make_identity(nc, ident[:])

            # resident small inputs
            spansT_sb = sb.tile([128, 6, 128], dt.float8e4)
            nc.sync.dma_start(
                spansT_sb[:],
                wb[:, NTILE * 6 * TC:].rearrange("p (a b) -> p a b", a=6))
            gidx_sb = sb.tile([128, 2, NIH // 16], dt.int16)
            nc.sync.dma_start(
                gidx_sb[:], gidx[:].rearrange("p (a b) -> p a b", a=2))
            aux_sb = sb.tile([128, AUXW], dt.float32)
            nc.sync.dma_start(aux_sb[:], aux[:])

            # ---------- phase 1: Vt[p, t] = sum_e spansT[e, p] wt[e, t] ----
            # split into two halves so half-0 gathers overlap half-1 matmuls
            vth = [sb.tile([128, H0], dt.float32, name="vt0"),
                   sb.tile([128, H1], dt.float32, name="vt1")]
            for tau in range(NTILE):
                w_t = wbp.tile([128, 6 * TC], dt.float8e4, tag="wt")
                nc.sync.dma_start(
                    w_t[:], wb[:, tau * 6 * TC:(tau + 1) * 6 * TC])
                ps = vps.tile([128, TC], dt.float32)
                for e in range(6):
                    nc.tensor.matmul(
                        ps[:], spansT_sb[:, e, :],
                        w_t[:, e * TC:(e + 1) * TC],
                        start=(e == 0), stop=(e == 5))
                h, off = ((0, tau * TC) if tau < NT0
                          else (1, (tau - NT0) * TC))
                nc.vector.tensor_copy(vth[h][:, off:off + TC], ps[:])

            # ---------- phase 2: gather + mask + bucket reduce -------------
            psum1 = sb.tile([128, 32], dt.float32)
            if phase < 2:
                nc.vector.tensor_copy(psum1[:], vth[0][:, 0:32])
                nc.vector.tensor_add(psum1[:], psum1[:], vth[1][:, 0:32])
            for h, nel in ((0, H0), (1, H1)) if phase >= 2 else ():
                g = sb.tile([128, NIH], dt.float32, tag=f"g{h}")
                nc.gpsimd.ap_gather(
                    out_ap=g[:], in_ap=vth[h][:], idxs_ap=gidx_sb[:, h, :],
                    channels=128, num_elems=nel, d=1, num_idxs=NIH)
                moff = OFF_MATT + h * NIH
                nc.vector.tensor_tensor(
                    out=g[:], in0=g[:], in1=aux_sb[:, moff:moff + NIH],
                    op=OP.mult)
                psc = sb.tile([128, 32], dt.float32, tag=f"psc{h}")
                nc.vector.tensor_reduce(
                    out=psc[:, :, None],
                    in_=g[:].rearrange("p (a b) -> p a b", a=32),
                    axis=AX.X, op=OP.add)
                if h == 0:
                    nc.vector.tensor_copy(psum1[:], psc[:])
                else:
                    nc.vector.tensor_add(psum1[:], psum1[:], psc[:])

            # ---------- phase 3: AllGather [128, 32] + local sum -----------
            # (AllGather avoids the collective unit's AllReduce surcharge)
            sum1 = sm.tile([128, 32], dt.float32)
            if phase >= 3:
                ag_in = dram.tile([128, 32], dt.float32)
                ag_out = dram.tile([N_CORES * 128, 32], dt.float32)
                nc.sync.dma_start(ag_in[:], psum1[:])
                nc.gpsimd.collective_compute(
                    "AllGather", OP.bypass, replica_groups=rg,
                    ins=[ag_in.opt()], outs=[ag_out.opt()])
                parts = sm.tile([128, 8, 32], dt.float32)
                nc.sync.dma_start(
                    parts[:], ag_out[:].rearrange("(a p) b -> p a b", a=8))
                p4 = sm.tile([128, 4, 32], dt.float32)
                nc.vector.tensor_tensor(
                    out=p4[:].rearrange("p a b -> p (a b)"),
                    in0=parts[:, 0:4, :].rearrange("p a b -> p (a b)"),
                    in1=parts[:, 4:8, :].rearrange("p a b -> p (a b)"),
                    op=OP.add)
                p2 = sm.tile([128, 2, 32], dt.float32)
                nc.vector.tensor_tensor(
                    out=p2[:].rearrange("p a b -> p (a b)"),
                    in0=p4[:, 0:2, :].rearrange("p a b -> p (a b)"),
                    in1=p4[:, 2:4, :].rearrange("p a b -> p (a b)"),
                    op=OP.add)
                nc.vector.tensor_tensor(
                    out=sum1[:], in0=p2[:, 0, :], in1=p2[:, 1, :], op=OP.add)
            else:
                nc.vector.tensor_copy(sum1[:], psum1[:])

            # ---------- phase 4: softmaxes ---------------------------------
            # span scores: ssc[p] = span_embs[p] . span_W + b
            tmp768 = sm.tile([128, E], dt.float32)
            nc.vector.tensor_tensor(
                out=tmp768[:], in0=aux_sb[:, OFF_SPAL:OFF_SPAL + E],
                in1=aux_sb[:, OFF_SPW:OFF_SPW + E], op=OP.mult)
            ssc = sm.tile([128, 1], dt.float32)
            nc.vector.tensor_reduce(out=ssc[:], in_=tmp768[:], axis=AX.X,
                                    op=OP.add)
            nc.vector.tensor_add(ssc[:], ssc[:],
                                 aux_sb[:, OFF_SPB:OFF_SPB + 1])

            # softmax over s' = j2 % 16 (strided view [128, 2, 16])
            def v216(ap):
                return ap.rearrange("p (two s2) -> p two s2", two=2)
            mx = sm.tile([128, 2], dt.float32)
            nc.vector.tensor_reduce(out=mx[:, :, None], in_=v216(sum1[:]),
                                    axis=AX.X, op=OP.max)
            e1 = sm.tile([128, 32], dt.float32)
            nc.vector.tensor_tensor(
                out=v216(e1[:]), in0=v216(sum1[:]),
                in1=mx[:, :, None].to_broadcast([128, 2, 16]), op=OP.subtract)
            nc.scalar.activation(e1[:], e1[:], ACT.Exp)
            smsum = sm.tile([128, 2], dt.float32)
            nc.vector.tensor_reduce(out=smsum[:, :, None], in_=v216(e1[:]),
                                    axis=AX.X, op=OP.add)
            rsm = sm.tile([128, 2], dt.float32)
            nc.vector.reciprocal(rsm[:], smsum[:])
            nc.vector.tensor_tensor(
                out=v216(e1[:]), in0=v216(e1[:]),
                in1=rsm[:, :, None].to_broadcast([128, 2, 16]), op=OP.mult)

            # SSB[p, j2] = span_score(b(p), j2 % 16) via hosted-mask matmul
            rhsb = sm.tile([128, 32], dt.float32)
            nc.vector.tensor_tensor(out=rhsb[:],
                                    in0=aux_sb[:, OFF_HM:OFF_HM + 32],
                                    in1=ssc[:].to_broadcast([128, 32]),
                                    op=OP.mult)
            ssb_ps = mps.tile([128, 32], dt.float32, tag="mm")
            nc.tensor.matmul(ssb_ps[:], aux_sb[:, OFF_HB:OFF_HB + 128],
                             rhsb[:], start=True, stop=True)
            mult2 = sm.tile([128, 32], dt.float32)
            nc.vector.tensor_tensor(out=mult2[:], in0=e1[:], in1=ssb_ps[:],
                                    op=OP.mult)

            # own-batch extraction -> [16, 32] -> [1, 512]
            own_ps = mps.tile([16, 32], dt.float32, tag="mm")
            nc.tensor.matmul(own_ps[:], aux_sb[:, OFF_HO:OFF_HO + 16],
                             mult2[:], start=True, stop=True)
            own = sm.tile([16, 32], dt.float32)
            nc.vector.tensor_copy(own[:], own_ps[:])
            cn = sm.tile([1, 512], dt.float32)
            nc.sync.dma_start(cn[:].rearrange("p (a bb) -> p a bb", a=16),
                              own[:])

            # softmax over 512
            mxn = sm.tile([1, 1], dt.float32)
            nc.vector.tensor_reduce(out=mxn[:], in_=cn[:], axis=AX.X,
                                    op=OP.max, negate=True)
            e5 = sm.tile([1, 512], dt.float32)
            nc.scalar.activation(e5[:], cn[:], ACT.Exp, bias=mxn[:], scale=1.0)
            s5 = sm.tile([1, 1], dt.float32)
            nc.vector.tensor_reduce(out=s5[:], in_=e5[:], axis=AX.X, op=OP.add)
            r5 = sm.tile([1, 1], dt.float32)
            nc.vector.reciprocal(r5[:], s5[:])
            cand = sm.tile([1, 512], dt.float32)
            nc.vector.tensor_tensor(out=cand[:], in0=e5[:],
                                    in1=r5[:].to_broadcast([1, 512]),
                                    op=OP.mult)

            # ---------- phase 5: duplicate resolution + output -------------
            ones128 = sm.tile([1, 128], dt.float32)
            nc.vector.memset(ones128[:], 1.0)
            cb_ps = mps.tile([128, 512], dt.float32, tag="mm")
            nc.tensor.matmul(cb_ps[:], ones128[:], cand[:], start=True,
                             stop=True)
            candB = sm.tile([128, 512], dt.float32)
            nc.vector.tensor_copy(candB[:], cb_ps[:])

            qfp = aux_sb[:, OFF_QF:OFF_QF + 4]
            qff = aux_sb[:, OFF_QFF:OFF_QFF + 512]
            qip = sm.tile([128, 4], dt.int32)
            nc.sync.dma_start(qip[:], qidp_i[:])

            eq = sm.tile([128, 4, 512], dt.float32)
            nc.vector.tensor_tensor(
                out=eq[:], in0=qfp[:, :, None].to_broadcast([128, 4, 512]),
                in1=qff[:, None, :].to_broadcast([128, 4, 512]),
                op=OP.is_equal)
            count = sm.tile([128, 4], dt.float32)
            nc.vector.tensor_reduce(out=count[:, :, None], in_=eq[:],
                                    axis=AX.X, op=OP.add)
            nc.vector.tensor_tensor(
                out=eq[:], in0=eq[:],
                in1=candB[:, None, :].to_broadcast([128, 4, 512]), op=OP.mult)
            dup = sm.tile([128, 4], dt.float32)
            nc.vector.tensor_reduce(out=dup[:, :, None], in_=eq[:],
                                    axis=AX.X, op=OP.add)

            mask = sm.tile([128, 4], dt.float32)
            nc.vector.tensor_scalar(out=mask[:], in0=qfp[:],
                                    scalar1=float(NE), scalar2=None,
                                    op0=OP.is_lt)
            rc = sm.tile([128, 4], dt.float32)
            nc.vector.reciprocal(rc[:], count[:])
            mrc = sm.tile([128, 4], dt.float32)
            nc.vector.tensor_tensor(out=mrc[:], in0=mask[:], in1=rc[:],
                                    op=OP.mult)
            md = sm.tile([128, 4], dt.float32)
            nc.vector.tensor_tensor(out=md[:], in0=dup[:], in1=mask[:],
                                    op=OP.mult)
            vmp = sm.tile([128, 1], dt.float32)
            nc.vector.tensor_reduce(out=vmp[:], in_=md[:], axis=AX.X,
                                    op=OP.max)
            nep = sm.tile([128, 1], dt.float32)
            nc.vector.tensor_reduce(out=nep[:], in_=mrc[:], axis=AX.X,
                                    op=OP.add)

            # cross-partition reductions via PE transpose
            tv_ps = tps.tile([128, 128], dt.float32, tag="tp")
            nc.tensor.transpose(tv_ps[:1, :], vmp[:], ident[:])
            tv = sm.tile([1, 128], dt.float32)
            nc.vector.tensor_copy(tv[:], tv_ps[:1, :])
            vmn = sm.tile([1, 1], dt.float32)
            nc.vector.tensor_reduce(out=vmn[:], in_=tv[:], axis=AX.X,
                                    op=OP.max, negate=True)   # -vmax
            tn_ps = tps.tile([128, 128], dt.float32, tag="tp")
            nc.tensor.transpose(tn_ps[:1, :], nep[:], ident[:])
            tn = sm.tile([1, 128], dt.float32)
            nc.vector.tensor_copy(tn[:], tn_ps[:1, :])
            neff = sm.tile([1, 1], dt.float32)
            nc.vector.tensor_reduce(out=neff[:], in_=tn[:], axis=AX.X,
                                    op=OP.add)

            vmn_ps = mps.tile([128, 1], dt.float32, tag="mm")
            nc.tensor.matmul(vmn_ps[:], ones128[:], vmn[:], start=True,
                             stop=True)
            vmnB = sm.tile([128, 1], dt.float32)
            nc.vector.tensor_copy(vmnB[:], vmn_ps[:])
            exd = sm.tile([128, 4], dt.float32)
            nc.scalar.activation(exd[:], dup[:], ACT.Exp, bias=vmnB[:],
                                 scale=1.0)

            sede = sm.tile([128, 4], dt.float32)
            nc.vector.tensor_tensor(out=sede[:], in0=mrc[:], in1=exd[:],
                                    op=OP.mult)
            sedp = sm.tile([128, 1], dt.float32)
            nc.vector.tensor_reduce(out=sedp[:], in_=sede[:], axis=AX.X,
                                    op=OP.add)
            ts_ps = tps.tile([128, 128], dt.float32, tag="tp")
            nc.tensor.transpose(ts_ps[:1, :], sedp[:], ident[:])
            tsed = sm.tile([1, 128], dt.float32)
            nc.vector.tensor_copy(tsed[:], ts_ps[:1, :])
            sed0 = sm.tile([1, 1], dt.float32)
            nc.vector.tensor_reduce(out=sed0[:], in_=tsed[:], axis=AX.X,
                                    op=OP.add)

            e_nm = sm.tile([1, 1], dt.float32)
            nc.scalar.activation(e_nm[:], vmn[:], ACT.Exp)     # exp(-vmax)
            t1 = sm.tile([1, 1], dt.float32)
            nc.vector.tensor_scalar(out=t1[:], in0=neff[:], scalar1=-1.0,
                                    scalar2=float(NE), op0=OP.mult,
                                    op1=OP.add)
            d1 = sm.tile([1, 1], dt.float32)
            nc.vector.tensor_tensor(out=d1[:], in0=t1[:], in1=e_nm[:],
                                    op=OP.mult)
            denom = sm.tile([1, 1], dt.float32)
            nc.vector.tensor_add(denom[:], d1[:], sed0[:])
            rden = sm.tile([1, 1], dt.float32)
            nc.vector.reciprocal(rden[:], denom[:])
            base = sm.tile([1, 1], dt.float32)
            nc.vector.tensor_tensor(out=base[:], in0=e_nm[:], in1=rden[:],
                                    op=OP.mult)

            br2 = sm.tile([1, 2], dt.float32)
            nc.vector.tensor_copy(br2[:, 0:1], rden[:])
            nc.vector.tensor_copy(br2[:, 1:2], base[:])
            bb_ps = mps.tile([128, 2], dt.float32, tag="mm")
            nc.tensor.matmul(bb_ps[:], ones128[:], br2[:], start=True,
                             stop=True)
            bb2 = sm.tile([128, 2], dt.float32)
            nc.vector.tensor_copy(bb2[:], bb_ps[:])

            outv = sm.tile([128, 4], dt.float32)
            nc.vector.tensor_tensor(out=outv[:], in0=exd[:],
                                    in1=bb2[:, 0:1].to_broadcast([128, 4]),
                                    op=OP.mult)

            fill = sm.tile([128, FILL_W], dt.float32)
            nc.vector.tensor_copy(fill[:],
                                  bb2[:, 1:2].to_broadcast([128, FILL_W]))
            for q in range(4):
                nc.sync.dma_start(out[:, q * FILL_W:(q + 1) * FILL_W],
                                  fill[:])
            tc.strict_bb_all_engine_barrier()
            out_flat = out[:].rearrange("p f -> (p f)")[:, None]
            nc.gpsimd.indirect_dma_start(
                out=out_flat,
                out_offset=bass.IndirectOffsetOnAxis(ap=qip[:], axis=0),
                in_=outv[:],
                in_offset=None)

    nc.compile()
    return nc


def _host_prep(span_embs, triplet_ids_tr, offsets_tr, attention_tr, qid_inds,
               emb_weight, span_W, span_b):
    span_embs = np.asarray(span_embs, dtype=np.float32)
    ids = np.asarray(triplet_ids_tr).astype(np.int64)
    offs = np.asarray(offsets_tr).astype(np.int64)
    att = np.asarray(attention_tr, dtype=np.float32)
    qid = np.asarray(qid_inds).astype(np.int64)
    emb_weight = np.asarray(emb_weight, dtype=np.float32)
    span_W = np.asarray(span_W, dtype=np.float32)
    span_b = np.asarray(span_b, dtype=np.float32)
    f8 = mybir.dt.np(mybir.dt.float8e4)

    # bag id per element (general sorted offsets, offs[b,0] == 0)
    pos = np.arange(L)
    seg = np.empty((B, L), dtype=np.int64)
    for b in range(B):
        seg[b] = np.searchsorted(offs[b], pos, side='right') - 1

    su = seg % 16                                 # span col / channel-in-group
    j2 = ((seg // 16) % 2) * 16 + seg // 32       # bucket (contiguous softmax)
    k_of = ids // TS
    lid = (ids % TS).astype(np.int64)
    half = (lid >= H0).astype(np.int64)
    lidx = lid - H0 * half                        # idx within half
    bidx = np.broadcast_to(np.arange(B)[:, None], (B, L))

    # rank within (core k, batch b, half, bucket j2), stable order
    key = (((k_of * B + bidx) * 2 + half) * 32 + j2).ravel()
    order = np.argsort(key, kind='stable')
    sk = key[order]
    starts = np.r_[0, np.flatnonzero(sk[1:] != sk[:-1]) + 1]
    group_id = np.cumsum(np.r_[0, (sk[1:] != sk[:-1]).astype(np.int64)])
    rank_sorted = np.arange(sk.size) - starts[group_id]
    rank = np.empty(sk.size, dtype=np.int64)
    rank[order] = rank_sorted

    NJ = max(96, ((int(rank.max()) + 1 + 7) // 8) * 8)   # slots per bucket
    NIH = 32 * NJ
    slot = (j2.ravel() * NJ + rank)

    kf = k_of.ravel()
    bf = bidx.ravel()
    hf = half.ravel()
    gidx_all = np.zeros((N_CORES, B, 2, NIH), dtype=np.int16)
    gidx_all[kf, bf, hf, slot] = lidx.ravel().astype(np.int16)
    matt_all = np.zeros((N_CORES, B, 16, 2, NIH), dtype=np.float32)
    matt_all[kf, bf, su.ravel(), hf, slot] = att.ravel() / (WSC * WSC)

    # wb: streaming W^T tiles + spansT, fp8 (W scaled into normal range)
    WT = emb_weight.T * WSC                        # [768, 100000] f32
    spans_all = np.ascontiguousarray(span_embs.reshape(128, E))
    spansT_blk = (spans_all.T * WSC).reshape(6, 128, 128).transpose(1, 0, 2) \
        .reshape(128, 768)

    spanw = np.tile(span_W[:, 0][None, :], (128, 1)).astype(np.float32)
    r = np.arange(128)
    hostb = (r[:, None] // 16 == r[None, :] // 16).astype(np.float32)
    hostm = (r[:, None] % 16 == np.arange(32)[None, :] % 16) \
        .astype(np.float32)

    x = np.arange(512)
    j2d = x % 32
    mx_map = x // 32 + 16 * (2 * (j2d % 16) + j2d // 16)   # position -> bag

    AUXW = OFF_MATT + 2 * NIH
    in_maps = []
    for k in range(N_CORES):
        wbk = np.empty((128, NTILE * 6 * TC + 768), dtype=f8)
        wtk = WT[:, k * TS:(k + 1) * TS]           # [768, 12500]
        wbk[:, :NTILE * 6 * TC] = (
            wtk.reshape(6, 128, NTILE, TC).transpose(1, 2, 0, 3)
            .reshape(128, NTILE * 6 * TC).astype(f8))
        wbk[:, NTILE * 6 * TC:] = spansT_blk.astype(f8)

        hostown = np.zeros((128, 16), dtype=np.float32)
        hostown[k * 16 + np.arange(16), np.arange(16)] = 1.0
        qx = qid[k][mx_map]

        auxk = np.zeros((128, AUXW), dtype=np.float32)
        auxk[:, OFF_SPAL:OFF_SPAL + E] = spans_all
        auxk[:, OFF_SPW:OFF_SPW + E] = spanw
        auxk[:, OFF_SPB] = float(span_b[0])
        auxk[:, OFF_HB:OFF_HB + 128] = hostb
        auxk[:, OFF_HM:OFF_HM + 32] = hostm
        auxk[:, OFF_HO:OFF_HO + 16] = hostown
        auxk[:, OFF_QF:OFF_QF + 4] = qx.reshape(128, 4)
        auxk[:, OFF_QFF:OFF_QFF + 512] = qx[None, :]
        auxk[:, OFF_MATT:] = matt_all[k].reshape(128, 2 * NIH)

        # wrap idx j -> partition 16b + j%16, free j//16 (per half)
        gk = np.zeros((128, 2, NIH // 16), dtype=np.int16)
        for b in range(B):
            for h in range(2):
                gk[b * 16:(b + 1) * 16, h, :] = \
                    gidx_all[k, b, h].reshape(NIH // 16, 16).T

        in_maps.append(dict(
            wb=wbk, aux=auxk, gidx=gk.reshape(128, 2 * (NIH // 16)),
            qidp_i=qx.reshape(128, 4).astype(np.int32),
        ))
    return in_maps, NJ


def kernel_run(inputs, trace=False):
    in_maps, NJ = _host_prep(**inputs)
    if NJ not in _cache:
        _cache[NJ] = _build(NJ)
    nc = _cache[NJ]
    res = run_bass_kernel_spmd(nc, in_maps, core_ids=list(range(N_CORES)),
                               trace=trace)
    out = np.stack([r["out"].reshape(-1)[:NE] for r in res.results])
    return out[:, :, None].astype(np.float32), res


def kernel(**inputs):
    out, _ = kernel_run(inputs)
    return out
